# revision 11
# baseline (speedup 1.0000x reference)
"""Trainium2 Bass kernel for nn_ConvNetLayer (GNN message passing layer).

Distribution across 8 NeuronCores:
 - bonds sharded contiguously (75000/core, padded to 75264), atoms sharded
   (37500/core, padded to 37888); atom feature table + weights replicated
 - edge phase computes edge_syn/edge_gates for the core's bond shard and
   contributes the gates shard to an AllGather so every core holds the
   full [NB, H] gates table for atom-phase gathers
 - BatchNorm batch stats via [128,2] AllReduce; a second pass applies BN
 - atom phase gathers neighbor atom rows + gate rows (indirect DMA),
   applies V/U on-chip, accumulates gated messages, BN + relu + residual

Compute layout is feature-major ([H on partitions, rows on free]) so the
BatchNorm/bias work rides ACT's per-partition scale/bias. Row-major <->
feature-major via PE transposes.

Padding invariants (keeps BN stats exact):
 - stats accumulate UNBIASED syn (bias folded back in later: BN(x+b) uses
   mean(x)+b and var(x))
 - pad bond rows are zero and their ex/ey gathers hit a zero row appended
   to the atom table => pad edge_syn == 0
 - pad atom rows are zero and their gate gathers hit a zero row appended
   to the gates table => pad atom_syn == 0
"""
import numpy as np

import jax
from jax.sharding import Mesh, PartitionSpec
from jax.experimental.shard_map import shard_map

import concourse.bass as bass
import concourse.mybir as mybir
import concourse.tile as tile
from concourse import bacc
from concourse.bass2jax import (
    _bass_exec_p, install_neuronx_cc_hook, partition_id_tensor,
)
from concourse.masks import make_identity

H = 128
NA = 300000
NB = 600000
NBRS = 6
N_CORES = 8
EPS = 1e-5

TB = 512                   # bonds per tile
TA = 512                   # atoms per tile
AF = mybir.ActivationFunctionType
ALU = mybir.AluOpType

_CACHE = {}


def _roundup(x, m):
    return (x + m - 1) // m * m


# ----------------------------------------------------------------------------
# program builder
# ----------------------------------------------------------------------------

def build_program(na=NA, nb=NB, n_cores=N_CORES):
    na_sh = _roundup(na // n_cores, TA)   # padded shard sizes
    nb_sh = _roundup(nb // n_cores, TB)
    na_ex = na + 128                      # atom table + zero rows
    nb_sh0 = nb // n_cores
    gpad = 16                             # zero rows appended per-core pre-AG
    nb_ex = (nb_sh0 + gpad) * n_cores     # gates table incl interleaved zeros
    nbt = nb_sh // TB
    nat = na_sh // TA
    cb = TB // 128
    ca = TA // 128

    nc = bacc.Bacc("TRN2", target_bir_lowering=False, debug=False,
                   num_devices=n_cores)
    dt = mybir.dt
    f32 = dt.float32
    bf16 = dt.bfloat16

    # ---- inputs ----
    atom_tbl = nc.dram_tensor("atom_tbl", [na_ex, H], f32, kind="ExternalInput")
    atom_sh = nc.dram_tensor("atom_sh", [na_sh, H], f32, kind="ExternalInput")
    bond_sh = nc.dram_tensor("bond_sh", [nb_sh, H], f32, kind="ExternalInput")
    ex_idx = nc.dram_tensor("ex_idx", [nbt, 128, cb], dt.int32, kind="ExternalInput")
    ey_idx = nc.dram_tensor("ey_idx", [nbt, 128, cb], dt.int32, kind="ExternalInput")
    nbr_idx = nc.dram_tensor("nbr_idx", [nat, NBRS, 128, ca], dt.int32,
                             kind="ExternalInput")
    gat_idx = nc.dram_tensor("gat_idx", [nat, NBRS, 128, ca], dt.int32,
                             kind="ExternalInput")
    wts = {}
    for nm in ("A", "B", "C", "U", "V"):
        wts[nm] = nc.dram_tensor(f"{nm}_wT", [H, H], f32, kind="ExternalInput")
    edge_bias = nc.dram_tensor("edge_bias", [H], f32, kind="ExternalInput")
    u_bias = nc.dram_tensor("u_bias", [H], f32, kind="ExternalInput")
    v_bias = nc.dram_tensor("v_bias", [H], f32, kind="ExternalInput")
    bn_b = nc.dram_tensor("bn_b", [2, H], f32, kind="ExternalInput")
    bn_a = nc.dram_tensor("bn_a", [2, H], f32, kind="ExternalInput")

    # ---- outputs (padded shards; host drops pad rows) ----
    bond_out = nc.dram_tensor("bond_out", [nb_sh, H], f32, kind="ExternalOutput")
    atom_out = nc.dram_tensor("atom_out", [na_sh, H], f32, kind="ExternalOutput")

    def rm_view(t, chunks):
        return t[:, :].rearrange("(n c p) f -> n p c f", p=128, c=chunks)

    atom_sh_v = rm_view(atom_sh, ca)
    bond_sh_v = rm_view(bond_sh, cb)
    bond_out_v = rm_view(bond_out, cb)
    atom_out_v = rm_view(atom_out, ca)

    with tile.TileContext(nc) as tc:
        with tc.tile_pool(name="const", bufs=1) as cpool, \
             tc.tile_pool(name="sb", bufs=3) as sb, \
             tc.tile_pool(name="sbT", bufs=3) as sbT, \
             tc.tile_pool(name="stats", bufs=1) as stp, \
             tc.tile_pool(name="ps_mm", bufs=2, space="PSUM") as ps_mm, \
             tc.tile_pool(name="ps_tr", bufs=3, space="PSUM") as ps_tr, \
             tc.tile_pool(name="dram", bufs=1, space="DRAM") as dram:

            # ---------------- constants ----------------
            ident = cpool.tile([128, 128], f32)
            make_identity(nc, ident[:])
            ident_bf = cpool.tile([128, 128], bf16)
            make_identity(nc, ident_bf[:])
            w_sb = {}
            for nm in ("A", "B", "C", "U", "V"):
                w_sb[nm] = cpool.tile([H, H], f32, name=f"w_{nm}")
                nc.sync.dma_start(out=w_sb[nm][:], in_=wts[nm][:, :])
            eb = cpool.tile([128, 1], f32)
            nc.sync.dma_start(out=eb[:], in_=edge_bias[:].rearrange("(p o) -> p o", o=1))
            ub = cpool.tile([128, 1], f32)
            nc.sync.dma_start(out=ub[:], in_=u_bias[:].rearrange("(p o) -> p o", o=1))
            vb = cpool.tile([128, 1], f32)
            nc.sync.dma_start(out=vb[:], in_=v_bias[:].rearrange("(p o) -> p o", o=1))
            bnb = cpool.tile([128, 2], f32)
            nc.sync.dma_start(out=bnb[:], in_=bn_b[:, :].rearrange("g p -> p g"))
            bna = cpool.tile([128, 2], f32)
            nc.sync.dma_start(out=bna[:], in_=bn_a[:, :].rearrange("g p -> p g"))
            eps_t = cpool.tile([128, 1], f32)
            nc.vector.memset(eps_t[:], EPS)
            zeros_t = cpool.tile([128, 128], bf16)
            nc.vector.memset(zeros_t[:], 0.0)

            # dram scratch
            syn_bond = dram.tile([nbt, 128, TB], f32)      # unbiased, feature-major
            syn_atom = dram.tile([nat, 128, TA], f32)
            gates_bounce = dram.tile([nb_sh0 + gpad, H], bf16)
            ag_gates = dram.tile([nb_ex, H], bf16, addr_space="Shared")
            gates_v = ag_gates[:, :]

            # zero rows at tail of this core's AG contribution
            nc.sync.dma_start(out=gates_bounce[nb_sh0:nb_sh0 + gpad, :],
                              in_=zeros_t[:gpad, :])

            s1b = stp.tile([128, nbt], f32)
            s2b = stp.tile([128, nbt], f32)
            s1a = stp.tile([128, nat], f32)
            s2a = stp.tile([128, nat], f32)

            def transpose_rm_to_fm(src_rm, chunks, name):
                idn = ident_bf if src_rm.dtype == bf16 else ident
                out_fm = sbT.tile([128, chunks * 128], f32, name=name, tag=name)
                for c in range(chunks):
                    pt = ps_tr.tile([128, 128], src_rm.dtype, name=f"{name}_ps",
                                    tag="trps")
                    nc.tensor.transpose(pt[:], src_rm[:, c, :], idn[:])
                    nc.scalar.copy(out_fm[:, c * 128:(c + 1) * 128], pt[:])
                return out_fm

            def gather_rm(table_ap, idx_sb, chunks, name, dtype=f32):
                g = sb.tile([128, chunks, 128], dtype, name=name, tag=name)
                for c in range(chunks):
                    nc.gpsimd.indirect_dma_start(
                        out=g[:, c, :],
                        out_offset=None,
                        in_=table_ap,
                        in_offset=bass.IndirectOffsetOnAxis(
                            ap=idx_sb[:, c:c + 1], axis=0),
                    )
                return g

            # ================= EDGE PHASE =================
            for t in range(nbt):
                bond_rm = sb.tile([128, cb, 128], f32, name="bond_rm", tag="bond_rm")
                nc.sync.dma_start(out=bond_rm[:], in_=bond_sh_v[t])
                exi = sb.tile([128, cb], dt.int32, name="exi", tag="exi")
                nc.sync.dma_start(out=exi[:], in_=ex_idx[t])
                eyi = sb.tile([128, cb], dt.int32, name="eyi", tag="eyi")
                nc.sync.dma_start(out=eyi[:], in_=ey_idx[t])
                ex_rm = gather_rm(atom_tbl[:, :], exi, cb, "ex_rm")
                ey_rm = gather_rm(atom_tbl[:, :], eyi, cb, "ey_rm")

                bond_fm = transpose_rm_to_fm(bond_rm, cb, "bond_fm")
                ex_fm = transpose_rm_to_fm(ex_rm, cb, "ex_fm")
                ey_fm = transpose_rm_to_fm(ey_rm, cb, "ey_fm")

                syn_ps = ps_mm.tile([128, TB], f32, name="syn_ps", tag="mmps")
                nc.tensor.matmul(syn_ps[:], w_sb["A"][:], bond_fm[:], start=True,
                                 stop=False)
                nc.tensor.matmul(syn_ps[:], w_sb["B"][:], ex_fm[:], start=False,
                                 stop=False)
                nc.tensor.matmul(syn_ps[:], w_sb["C"][:], ey_fm[:], start=False,
                                 stop=True)

                # unbiased syn -> stats + store; gates = sigmoid(syn + eb)
                syn_sb = sb.tile([128, TB], f32, name="syn_sb", tag="syn_sb")
                nc.scalar.activation(syn_sb[:], syn_ps[:], AF.Identity,
                                     accum_out=s1b[:, t:t + 1])
                sq = sb.tile([128, TB], f32, name="sq", tag="sq")
                nc.scalar.activation(sq[:], syn_sb[:], AF.Square,
                                     accum_out=s2b[:, t:t + 1])
                nc.sync.dma_start(out=syn_bond[t], in_=syn_sb[:])

                gate_fm = sb.tile([128, TB], f32, name="gate_fm", tag="gate_fm")
                nc.scalar.activation(gate_fm[:], syn_sb[:], AF.Sigmoid, bias=eb[:])
                g_rm = sb.tile([128, cb, 128], bf16, name="g_rm", tag="g_rm")
                for c in range(cb):
                    pt = ps_tr.tile([128, 128], f32, name="g_ps", tag="trps")
                    nc.tensor.transpose(pt[:], gate_fm[:, c * 128:(c + 1) * 128],
                                        ident[:])
                    nc.scalar.copy(g_rm[:, c, :], pt[:])
                # only the real (unpadded) rows go to the AG input
                for c in range(cb):
                    row0 = t * TB + c * 128
                    if row0 >= nb_sh0:
                        break
                    rows = min(128, nb_sh0 - row0)
                    nc.sync.dma_start(
                        out=gates_bounce[row0:row0 + rows, :],
                        in_=g_rm[:rows, c, :])

            # ---- bond BN stats + allreduce ----
            sb1 = stp.tile([128, 2], f32)
            nc.vector.tensor_reduce(sb1[:, 0:1], s1b[:], axis=mybir.AxisListType.X,
                                    op=ALU.add)
            nc.vector.tensor_reduce(sb1[:, 1:2], s2b[:], axis=mybir.AxisListType.X,
                                    op=ALU.add)
            arb_in = dram.tile([128, 2], f32)
            arb_out = dram.tile([128, 2], f32, addr_space="Shared")
            nc.sync.dma_start(out=arb_in[:, :], in_=sb1[:])
            nc.gpsimd.collective_compute(
                "AllReduce", ALU.add, replica_groups=[list(range(n_cores))],
                ins=[arb_in[:, :].opt()], outs=[arb_out[:, :].opt()])

            # gates allgather
            nc.gpsimd.collective_compute(
                "AllGather", ALU.bypass, replica_groups=[list(range(n_cores))],
                ins=[gates_bounce[:, :].opt()], outs=[ag_gates[:, :].opt()])

            def bn_coeffs(ar_out_dram, n_rows, gb_sb, name):
                """stats of UNBIASED x (BN is shift-invariant). -> scale, bias"""
                s12 = sb.tile([128, 2], f32, name=f"{name}_s12", tag=f"{name}_s12")
                nc.sync.dma_start(out=s12[:], in_=ar_out_dram[:, :])
                mean0 = sb.tile([128, 1], f32, name=f"{name}_mean0", tag=f"{name}_m0")
                nc.scalar.mul(mean0[:], s12[:, 0:1], 1.0 / n_rows)
                ex2 = sb.tile([128, 1], f32, name=f"{name}_ex2", tag=f"{name}_ex2")
                nc.scalar.mul(ex2[:], s12[:, 1:2], 1.0 / n_rows)
                m2 = sb.tile([128, 1], f32, name=f"{name}_m2", tag=f"{name}_m2")
                nc.vector.tensor_tensor(out=m2[:], in0=mean0[:], in1=mean0[:],
                                        op=ALU.mult)
                var = sb.tile([128, 1], f32, name=f"{name}_var", tag=f"{name}_var")
                nc.vector.tensor_tensor(out=var[:], in0=ex2[:], in1=m2[:],
                                        op=ALU.subtract)
                std = sb.tile([128, 1], f32, name=f"{name}_std", tag=f"{name}_std")
                nc.scalar.activation(std[:], var[:], AF.Sqrt, bias=eps_t[:])
                istd = sb.tile([128, 1], f32, name=f"{name}_istd", tag=f"{name}_istd")
                nc.vector.reciprocal(istd[:], std[:])
                scale = sb.tile([128, 1], f32, name=f"{name}_scale",
                                tag=f"{name}_scale")
                nc.vector.tensor_tensor(out=scale[:], in0=gb_sb[:, 0:1], in1=istd[:],
                                        op=ALU.mult)
                mg = sb.tile([128, 1], f32, name=f"{name}_mg", tag=f"{name}_mg")
                nc.vector.tensor_tensor(out=mg[:], in0=mean0[:], in1=scale[:],
                                        op=ALU.mult)
                bias = sb.tile([128, 1], f32, name=f"{name}_bias", tag=f"{name}_bias")
                nc.vector.tensor_tensor(out=bias[:], in0=gb_sb[:, 1:2], in1=mg[:],
                                        op=ALU.subtract)
                return scale, bias

            bscale, bbias = bn_coeffs(arb_out, nb, bnb, "bnb")

            # ---- bond BN apply pass ----
            for t in range(nbt):
                synt = sb.tile([128, TB], f32, name="synt", tag="synt")
                nc.sync.dma_start(out=synt[:], in_=syn_bond[t])
                rl = sb.tile([128, TB], f32, name="rl", tag="rl")
                nc.scalar.activation(rl[:], synt[:], AF.Relu,
                                     bias=bbias[:], scale=bscale[:])
                bo_rm = sb.tile([128, cb, 128], f32, name="bo_rm", tag="bo_rm")
                for c in range(cb):
                    pt = ps_tr.tile([128, 128], f32, name="bo_ps", tag="trps")
                    nc.tensor.transpose(pt[:], rl[:, c * 128:(c + 1) * 128], ident[:])
                    nc.scalar.copy(bo_rm[:, c, :], pt[:])
                nc.sync.dma_start(out=bond_out_v[t], in_=bo_rm[:])

            # ================= ATOM PHASE =================
            for t in range(nat):
                atom_rm = sb.tile([128, ca, 128], f32, name="atom_rm", tag="atom_rm")
                nc.sync.dma_start(out=atom_rm[:], in_=atom_sh_v[t])
                atom_fm = transpose_rm_to_fm(atom_rm, ca, "atom_fm")
                u_ps = ps_mm.tile([128, TA], f32, name="u_ps", tag="ups")
                nc.tensor.matmul(u_ps[:], w_sb["U"][:], atom_fm[:], start=True,
                                 stop=True)

                msg = sbT.tile([128, TA], f32, name="msg", tag="msg")
                for k in range(NBRS):
                    nbi = sb.tile([128, ca], dt.int32, name="nbi", tag="nbi")
                    nc.sync.dma_start(out=nbi[:], in_=nbr_idx[t, k])
                    gti = sb.tile([128, ca], dt.int32, name="gti", tag="gti")
                    nc.sync.dma_start(out=gti[:], in_=gat_idx[t, k])
                    nbr_rm = gather_rm(atom_tbl[:, :], nbi, ca, "nbr_rm")
                    gat_rm = gather_rm(gates_v, gti, ca, "gat_rm", dtype=bf16)
                    nbr_fm = transpose_rm_to_fm(nbr_rm, ca, "nbr_fm")
                    gat_fm = transpose_rm_to_fm(gat_rm, ca, "gat_fm")
                    v_ps = ps_mm.tile([128, TA], f32, name="v_ps", tag="mmps")
                    nc.tensor.matmul(v_ps[:], w_sb["V"][:], nbr_fm[:], start=True,
                                     stop=True)
                    vsb = sb.tile([128, TA], f32, name="vsb", tag="vsb")
                    nc.scalar.activation(vsb[:], v_ps[:], AF.Identity, bias=vb[:])
                    if k == 0:
                        nc.vector.tensor_tensor(out=msg[:], in0=gat_fm[:],
                                                in1=vsb[:], op=ALU.mult)
                    else:
                        prod = sb.tile([128, TA], f32, name="prod", tag="prod")
                        nc.vector.tensor_tensor(out=prod[:], in0=gat_fm[:],
                                                in1=vsb[:], op=ALU.mult)
                        nc.vector.tensor_tensor(out=msg[:], in0=msg[:],
                                                in1=prod[:], op=ALU.add)

                # unbiased atom syn (U bias folded into BN shift later)
                syna = sb.tile([128, TA], f32, name="syna", tag="syna")
                nc.vector.scalar_tensor_tensor(
                    out=syna[:], in0=u_ps[:], scalar=0.0, in1=msg[:],
                    op0=ALU.add, op1=ALU.add, accum_out=s1a[:, t:t + 1])
                sqa = sb.tile([128, TA], f32, name="sqa", tag="sqa")
                nc.scalar.activation(sqa[:], syna[:], AF.Square,
                                     accum_out=s2a[:, t:t + 1])
                nc.sync.dma_start(out=syn_atom[t], in_=syna[:])

            # ---- atom BN stats + allreduce ----
            sa1 = stp.tile([128, 2], f32)
            nc.vector.tensor_reduce(sa1[:, 0:1], s1a[:], axis=mybir.AxisListType.X,
                                    op=ALU.add)
            nc.vector.tensor_reduce(sa1[:, 1:2], s2a[:], axis=mybir.AxisListType.X,
                                    op=ALU.add)
            ara_in = dram.tile([128, 2], f32)
            ara_out = dram.tile([128, 2], f32, addr_space="Shared")
            nc.sync.dma_start(out=ara_in[:, :], in_=sa1[:])
            nc.gpsimd.collective_compute(
                "AllReduce", ALU.add, replica_groups=[list(range(n_cores))],
                ins=[ara_in[:, :].opt()], outs=[ara_out[:, :].opt()])
            ascale, abias = bn_coeffs(ara_out, na, bna, "bna")

            # ---- atom BN apply + residual pass ----
            for t in range(nat):
                synt = sb.tile([128, TA], f32, name="synat", tag="synat")
                nc.sync.dma_start(out=synt[:], in_=syn_atom[t])
                rl = sb.tile([128, TA], f32, name="rla", tag="rla")
                nc.scalar.activation(rl[:], synt[:], AF.Relu,
                                     bias=abias[:], scale=ascale[:])
                res_rm = sb.tile([128, ca, 128], f32, name="res_rm", tag="res_rm")
                nc.sync.dma_start(out=res_rm[:], in_=atom_sh_v[t])
                ao_rm = sb.tile([128, ca, 128], f32, name="ao_rm", tag="ao_rm")
                for c in range(ca):
                    pt = ps_tr.tile([128, 128], f32, name="ao_ps", tag="trps")
                    nc.tensor.transpose(pt[:], rl[:, c * 128:(c + 1) * 128], ident[:])
                    nc.vector.tensor_tensor(out=ao_rm[:, c, :], in0=pt[:],
                                            in1=res_rm[:, c, :], op=ALU.add)
                nc.sync.dma_start(out=atom_out_v[t], in_=ao_rm[:])

    nc.compile()
    return nc


# ----------------------------------------------------------------------------
# host-side input prep
# ----------------------------------------------------------------------------

def prep_inputs(inputs, na=NA, nb=NB, n_cores=N_CORES):
    na_sh0, nb_sh0 = na // n_cores, nb // n_cores
    na_sh, nb_sh = _roundup(na_sh0, TA), _roundup(nb_sh0, TB)
    nbt, nat = nb_sh // TB, na_sh // TA
    cb, ca = TB // 128, TA // 128

    atom = np.asarray(inputs["atom_layer_input"], dtype=np.float32)
    bond = np.asarray(inputs["bond_layer_input"], dtype=np.float32)
    aag = np.asarray(inputs["atom_adjacency_graph"], dtype=np.int32)
    abag = np.asarray(inputs["atom_bond_adjacency_graph"], dtype=np.int32)
    bag = np.asarray(inputs["bond_atom_adjacency_graph"], dtype=np.int32)

    atom_ex = np.concatenate([atom, np.zeros((128, H), np.float32)], axis=0)

    wT = {nm: np.ascontiguousarray(np.asarray(inputs[f"{nm}_w"], np.float32).T)
          for nm in ("A", "B", "C", "U", "V")}
    edge_bias = (np.asarray(inputs["A_b"]) + np.asarray(inputs["B_b"])
                 + np.asarray(inputs["C_b"])).astype(np.float32)
    bn_b = np.stack([np.asarray(inputs["bn_bond_gamma"], np.float32),
                     np.asarray(inputs["bn_bond_beta"], np.float32)])
    bn_a = np.stack([np.asarray(inputs["bn_atom_gamma"], np.float32),
                     np.asarray(inputs["bn_atom_beta"], np.float32)])

    def pad_rows(x, n, fill=0):
        if x.shape[0] == n:
            return x
        pad = np.full((n - x.shape[0],) + x.shape[1:], fill, x.dtype)
        return np.concatenate([x, pad], axis=0)

    def tile_idx_bond(idx_flat):
        return np.ascontiguousarray(
            idx_flat.reshape(nbt, cb, 128).transpose(0, 2, 1))

    in_maps = []
    for c in range(n_cores):
        b0, b1 = c * nb_sh0, (c + 1) * nb_sh0
        a0, a1 = c * na_sh0, (c + 1) * na_sh0
        # bond shard: pad rows zero; pad gather idx -> zero row (na)
        ex_pad = pad_rows(bag[b0:b1, 0], nb_sh, fill=na).astype(np.int32)
        ey_pad = pad_rows(bag[b0:b1, 1], nb_sh, fill=na).astype(np.int32)
        # atom shard: pad nbr idx -> row 0 (killed by zero gate), gate idx ->
        # zero row (nb)
        aag_pad = pad_rows(aag[a0:a1], na_sh, fill=0).astype(np.int32)
        # remap bond ids into the AG layout with 16 zero rows per core block,
        # pads point at core 0's zero row (= row nb_sh0)
        abag_rm = abag[a0:a1] + (abag[a0:a1] // nb_sh0) * 16
        abag_pad = pad_rows(abag_rm, na_sh, fill=nb_sh0).astype(np.int32)
        nbr_idx = np.ascontiguousarray(
            aag_pad.reshape(nat, ca, 128, NBRS).transpose(0, 3, 2, 1))
        gat_idx = np.ascontiguousarray(
            abag_pad.reshape(nat, ca, 128, NBRS).transpose(0, 3, 2, 1))
        m = {
            "atom_tbl": atom_ex,
            "atom_sh": pad_rows(atom[a0:a1], na_sh),
            "bond_sh": pad_rows(bond[b0:b1], nb_sh),
            "ex_idx": tile_idx_bond(ex_pad),
            "ey_idx": tile_idx_bond(ey_pad),
            "nbr_idx": nbr_idx, "gat_idx": gat_idx,
            "edge_bias": edge_bias,
            "u_bias": np.asarray(inputs["U_b"], np.float32),
            "v_bias": np.asarray(inputs["V_b"], np.float32),
            "bn_b": bn_b, "bn_a": bn_a,
        }
        for nm in ("A", "B", "C", "U", "V"):
            m[f"{nm}_wT"] = wT[nm]
        in_maps.append(m)
    return in_maps


# ----------------------------------------------------------------------------
# SPMD runner (jit built once, reusable)
# ----------------------------------------------------------------------------

class SpmdRunner:
    def __init__(self, nc, n_cores):
        install_neuronx_cc_hook()
        self.nc = nc
        self.n_cores = n_cores
        pname = nc.partition_id_tensor.name if nc.partition_id_tensor else None
        in_names, out_names, out_avals, zero_outs = [], [], [], []
        for alloc in nc.m.functions[0].allocations:
            if not isinstance(alloc, mybir.MemoryLocationSet):
                continue
            name = alloc.memorylocations[0].name
            if alloc.kind == "ExternalInput":
                if name != pname:
                    in_names.append(name)
            elif alloc.kind == "ExternalOutput":
                shape = tuple(alloc.tensor_shape)
                dtype = mybir.dt.np(alloc.dtype)
                out_names.append(name)
                out_avals.append(jax.core.ShapedArray(shape, dtype))
                zero_outs.append(np.zeros(shape, dtype))
        self.in_names, self.out_names = in_names, out_names
        self.out_avals, self.zero_outs = out_avals, zero_outs
        n_params, n_outs = len(in_names), len(out_avals)
        all_in = list(in_names) + list(out_names)
        if pname is not None:
            all_in.append(pname)

        def _body(*args):
            operands = list(args)
            if pname is not None:
                operands.append(partition_id_tensor())
            return tuple(_bass_exec_p.bind(
                *operands, out_avals=tuple(out_avals), in_names=tuple(all_in),
                out_names=tuple(out_names), lowering_input_output_aliases=(),
                sim_require_finite=False, sim_require_nnan=False, nc=nc))

        devices = jax.devices()[:n_cores]
        self.mesh = Mesh(np.asarray(devices), ("core",))
        specs = (PartitionSpec("core"),) * (n_params + n_outs)
        self.fn = jax.jit(
            shard_map(_body, mesh=self.mesh, in_specs=specs,
                      out_specs=(PartitionSpec("core"),) * n_outs,
                      check_rep=False),
            keep_unused=True)
        self.sharding = jax.sharding.NamedSharding(self.mesh, PartitionSpec("core"))

    def stage_inputs(self, in_maps):
        staged = []
        for name in self.in_names:
            arr = np.concatenate([np.asarray(m[name]) for m in in_maps], axis=0)
            staged.append(jax.device_put(arr, self.sharding))
        for z in self.zero_outs:
            zz = np.zeros((self.n_cores * z.shape[0], *z.shape[1:]), z.dtype)
            staged.append(jax.device_put(zz, self.sharding))
        return staged

    def run(self, staged):
        outs = self.fn(*staged)
        jax.block_until_ready(outs)
        return outs


# ----------------------------------------------------------------------------
# public entry point
# ----------------------------------------------------------------------------

def _get_runner():
    if "prog" not in _CACHE:
        nc = build_program()
        _CACHE["prog"] = SpmdRunner(nc, N_CORES)
    return _CACHE["prog"]


def kernel(**inputs):
    r = _get_runner()
    in_maps = prep_inputs(inputs)
    staged = r.stage_inputs(in_maps)
    outs = r.run(staged)

    na_sh0, nb_sh0 = NA // N_CORES, NB // N_CORES
    na_sh, nb_sh = _roundup(na_sh0, TA), _roundup(nb_sh0, TB)
    atom_full = np.asarray(outs[r.out_names.index("atom_out")]) \
        .reshape(N_CORES, na_sh, H)[:, :na_sh0].reshape(NA, H)
    bond_full = np.asarray(outs[r.out_names.index("bond_out")]) \
        .reshape(N_CORES, nb_sh, H)[:, :nb_sh0].reshape(NB, H)
    return (atom_full, bond_full)
